# revision 1
# baseline (speedup 1.0000x reference)
"""AudioMamba2 fused TRN2 kernel: 8-core data-parallel Bass/Tile.

Self-contained: host folds weights, transposes x to bf16 xT[37,N] with a
baked ones row, runs a two-phase (silu / exp-ln table set) row-major
pipeline per core, returns the full [N, 32] softmax output.
"""
import numpy as np
import ml_dtypes
from contextlib import ExitStack

import concourse.bass as bass
import concourse.mybir as mybir
import concourse.tile as tile
from concourse.bass_types import AP

F32 = mybir.dt.float32
BF16 = mybir.dt.bfloat16
AF = mybir.ActivationFunctionType
ALU = mybir.AluOpType

IN_DIM = 36
D_MODEL = 32
D_INNER = 64
D_STATE = 8
NHEADS = 8
HEADDIM = 8
CONV_DIM = 80
D_IN_PROJ = 152
NORM_EPS = 1e-5
K1 = 37          # 36 features + ones row
NSIL = 144       # z|xh|B|C channels (silu'd)


def fold_weights(f_out_w, f_out_b, in_proj_w, conv_w, conv_b, dt_bias,
                 A_log, D_skip, norm_w, out_proj_w):
    f64 = np.float64
    W12 = in_proj_w.astype(f64) @ f_out_w.astype(f64)          # [152, 36]
    b12 = in_proj_w.astype(f64) @ f_out_b.astype(f64)          # [152]
    s80 = conv_w[:, -1].astype(f64)
    W12[64:144] *= s80[:, None]
    b12[64:144] = b12[64:144] * s80 + conv_b.astype(f64)
    b12[144:152] += dt_bias.astype(f64)
    W1 = np.concatenate([W12, b12[:, None]], axis=1)           # [152, 37]
    W1T = np.ascontiguousarray(W1.T)                           # [37, 152]
    Wout = out_proj_w.astype(f64) * norm_w.astype(f64)[None, :]  # [32, 64]
    WoutT = np.ascontiguousarray(Wout.T)                       # [64, 32]
    WoutT2 = np.concatenate([WoutT, WoutT], axis=0)            # [128, 32]
    return (W1T.astype(ml_dtypes.bfloat16),
            WoutT2.astype(ml_dtypes.bfloat16),
            np.ascontiguousarray(
                np.broadcast_to(D_skip.astype(np.float32), (128, 8))))


def prep_xt(x):
    """x [N, 36] f32 -> xT [37, N] bf16 with ones row."""
    N = x.shape[0]
    xt = np.empty((K1, N), dtype=ml_dtypes.bfloat16)
    xt[:IN_DIM] = x.T.astype(ml_dtypes.bfloat16)
    xt[IN_DIM] = np.float32(1.0)
    return xt


def bcast(ap, count):
    """Append a step-0 innermost free dim of size `count` to an AP."""
    return AP(ap.tensor, ap.offset, list(ap.ap) + [[0, count]])


def build_kernel(npc, num_cores=8, y_engine="vector", sim_safe=False,
                 debug_stop=None):
    """Build the Bass program for one core processing npc rows."""
    assert npc % 128 == 0
    NB = npc // 128
    nc = bass.Bass("TRN2", target_bir_lowering=False, num_devices=num_cores)

    xt_d = nc.dram_tensor("xt", [K1, npc], BF16, kind="ExternalInput")
    w1t_d = nc.dram_tensor("w1t", [K1, D_IN_PROJ], BF16, kind="ExternalInput")
    woutt_d = nc.dram_tensor("woutt", [128, D_MODEL], BF16,
                             kind="ExternalInput")
    db_d = nc.dram_tensor("db", [128, NHEADS], F32, kind="ExternalInput")
    eps_d = nc.dram_tensor("eps", [128, 1], F32, kind="ExternalInput")
    id_d = nc.dram_tensor("ident", [128, 128], BF16, kind="ExternalInput")
    out_d = nc.dram_tensor("out", [npc, D_MODEL], F32, kind="ExternalOutput")

    # persistent SBUF stores
    w1t_s = nc.alloc_sbuf_tensor("w1t_s", [K1, D_IN_PROJ], BF16)
    woutt_s = nc.alloc_sbuf_tensor("woutt_s", [128, D_MODEL], BF16)
    db_s = nc.alloc_sbuf_tensor("db_s", [128, NHEADS], F32)
    eps_s = nc.alloc_sbuf_tensor("eps_s", [128, 1], F32)
    id_s = nc.alloc_sbuf_tensor("id_s", [128, 128], BF16)
    t1_st = nc.alloc_sbuf_tensor("t1_st", [128, NB, D_INNER], BF16)
    w_st = nc.alloc_sbuf_tensor("w_st", [128, NB, NHEADS], F32)
    bc_st = nc.alloc_sbuf_tensor("bc_st", [128, NB], F32)
    ss_st = nc.alloc_sbuf_tensor("ss_st", [128, NB], F32)
    r_st = nc.alloc_sbuf_tensor("r_st", [128, NB], F32)

    XCH = 64            # x-in DMA chunk, blocks
    GP = 3              # PSUM silu-group
    GW = 24             # w (dt-preact) PSUM group (multiple of GP)
    GB = 16             # DVE batch group (beta)
    GD = 16             # dt/r batch group
    GO = 16             # out2/exp group

    ve = nc.vector
    ye = {"vector": nc.vector, "gpsimd": nc.gpsimd}[y_engine]

    with tile.TileContext(nc) as tc:
        # one-time const loads
        nc.sync.dma_start(w1t_s.ap(), w1t_d.ap())
        nc.sync.dma_start(woutt_s.ap(), woutt_d.ap())
        nc.sync.dma_start(db_s.ap(), db_d.ap())
        nc.sync.dma_start(eps_s.ap(), eps_d.ap())
        nc.sync.dma_start(id_s.ap(), id_d.ap())

        with (
            tc.tile_pool(name="xtp", bufs=3) as xtp,
            tc.tile_pool(name="pa", bufs=5, space="PSUM") as pap,
            tc.tile_pool(name="wps", bufs=2, space="PSUM") as wpsp,
            tc.tile_pool(name="sp", bufs=6) as sp,
            tc.tile_pool(name="prp", bufs=3) as prp,
        ):
            # ---------------- phase A ----------------
            xt_tiles = {}
            w_ps = None
            blocks = list(range(NB))
            groups = [blocks[i:i + GP] for i in range(0, NB, GP)]
            for grp in groups:
                g0 = grp[0]
                for b in grp:
                    ci = b // XCH
                    if ci not in xt_tiles:
                        t = xtp.tile([K1, XCH * 128], BF16)
                        c0 = ci * XCH
                        nc.sync.dma_start(
                            t[:, : min(XCH, NB - c0) * 128],
                            xt_d[:, c0 * 128: min(c0 + XCH, NB) * 128])
                        xt_tiles[ci] = t
                if g0 % GW == 0:
                    w_ps = wpsp.tile([128, GW * NHEADS], F32)
                ng = len(grp)
                P = pap.tile([128, GP * NSIL], F32)
                for j, b in enumerate(grp):
                    xt_sl = xt_tiles[b // XCH][
                        :, (b % XCH) * 128: (b % XCH) * 128 + 128]
                    nc.tensor.matmul(P[:, j * NSIL:(j + 1) * NSIL],
                                     xt_sl, w1t_s[:, 0:NSIL])
                    nc.tensor.matmul(
                        w_ps[:, (b % GW) * NHEADS:(b % GW + 1) * NHEADS],
                        xt_sl, w1t_s[:, NSIL:D_IN_PROJ])
                S = sp.tile([128, GP, NSIL], BF16)
                Pv = P.rearrange("p (g c) -> p g c", c=NSIL)[:, :ng, :]
                if sim_safe:
                    # CoreSim lacks Silu: sigmoid + explicit mul
                    nc.scalar.activation(S[:, :ng, :], Pv, AF.Sigmoid)
                    ve.tensor_tensor(out=S[:, :ng, :], in0=S[:, :ng, :],
                                     in1=Pv, op=ALU.mult)
                else:
                    nc.scalar.activation(S[:, :ng, :], Pv, AF.Silu)
                # t1 = S_z * S_xh
                ve.tensor_tensor(
                    out=t1_st[:, g0:g0 + ng, :],
                    in0=S[:, :ng, 0:64], in1=S[:, :ng, 64:128],
                    op=ALU.mult)
                # bc = sum(S_B * S_C)
                pr = prp.tile([128, GP, D_STATE], BF16)
                ve.tensor_tensor(out=pr[:, :ng, :],
                                 in0=S[:, :ng, 128:136], in1=S[:, :ng, 136:144],
                                 op=ALU.mult)
                ve.tensor_reduce(out=bc_st[:, g0:g0 + ng], in_=pr[:, :ng, :],
                                 axis=mybir.AxisListType.X, op=ALU.add)
                if (g0 + ng) % GW == 0 or (g0 + ng) == NB:
                    wg0 = (g0 + ng - 1) // GW * GW
                    nw = g0 + ng - wg0
                    nc.scalar.activation(
                        w_st[:, wg0:wg0 + nw, :],
                        w_ps.rearrange("p (g c) -> p g c", c=NHEADS)[:, :nw, :],
                        AF.Copy)

        if debug_stop == "a":
            with tc.tile_pool(name="zp", bufs=1) as zp:
                z = zp.tile([128, NB, D_MODEL], F32)
                nc.vector.memset(z, 0.0)
                nc.sync.dma_start(
                    out_d.rearrange("(nb p) c -> p nb c", p=128), z)
            return nc

        # ---------------- phase B ----------------
        with (
            tc.tile_pool(name="dtp", bufs=3) as dtp,
            tc.tile_pool(name="fp", bufs=3) as fp,
            tc.tile_pool(name="yp", bufs=3) as yp,
            tc.tile_pool(name="ytpp", bufs=3, space="PSUM") as ytpp,
            tc.tile_pool(name="ytp", bufs=4) as ytp,
            tc.tile_pool(name="sqp", bufs=2) as sqp,
            tc.tile_pool(name="o2p", bufs=3, space="PSUM") as o2p,
            tc.tile_pool(name="onp", bufs=2) as onp,
            tc.tile_pool(name="ep", bufs=3) as ep,
            tc.tile_pool(name="sep", bufs=2) as sep,
            tc.tile_pool(name="osp", bufs=3) as osp,
        ):
            zp_ctx = None
            for m0 in range(0, NB, GD):     # 16-block macro
                nm = min(GD, NB - m0)
                # softplus: dt = ln(1 + exp(w))
                dt_t = dtp.tile([128, GD, NHEADS], F32)
                nc.scalar.activation(dt_t[:, :nm, :], w_st[:, m0:m0 + nm, :],
                                     AF.Exp)
                nc.scalar.activation(dt_t[:, :nm, :], dt_t[:, :nm, :],
                                     AF.Ln, bias=1.0)
                yt_tiles = []
                for q0 in range(m0, m0 + nm, GB):
                    nq = min(GB, NB - q0)
                    f4 = fp.tile([128, GB, NHEADS], F32)
                    # dtbc = dt * bc_b ; f4 = dtbc + D_b
                    ve.tensor_tensor(
                        out=f4[:, :nq, :],
                        in0=dt_t[:, q0 - m0:q0 - m0 + nq, :],
                        in1=bcast(bc_st[:, q0:q0 + nq], NHEADS),
                        op=ALU.mult)
                    ve.tensor_tensor(
                        out=f4[:, :nq, :], in0=f4[:, :nq, :],
                        in1=AP(db_s.ap().tensor, 0,
                               [[NHEADS, 128], [0, GB], [1, NHEADS]])[:, :nq, :],
                        op=ALU.add)
                    # y_u = t1 * f4_b   (bf16, pair layout for xbar)
                    yu = yp.tile([128, GB * D_INNER], BF16)
                    ye.tensor_tensor(
                        out=yu.rearrange("p (g c) -> p g c", c=D_INNER)[:, :nq, :],
                        in0=t1_st[:, q0:q0 + nq, :]
                            .rearrange("p g (h d) -> p g h d", d=HEADDIM),
                        in1=bcast(f4[:, :nq, :], HEADDIM),
                        op=ALU.mult)
                    # transpose pairs -> yT (PE transpose + PSUM->SBUF copy)
                    for pi in (range(0, nq, 2) if debug_stop not in ("b1",) else []):
                        ytps = ytpp.tile([128, 128], BF16)
                        nc.tensor.transpose(ytps, yu[:, pi * 64:(pi + 2) * 64],
                                            id_s.ap())
                        ytt = ytp.tile([128, 128], BF16)
                        if (pi // 2) % 2 == 0:
                            nc.scalar.copy(ytt, ytps)
                        else:
                            ve.tensor_copy(ytt, ytps)
                        yt_tiles.append(ytt)
                    # ss = sum(y_u^2)
                    sq = sqp.tile([128, GB, D_INNER], BF16)
                    ve.tensor_tensor(
                        out=sq[:, :nq, :],
                        in0=yu.rearrange("p (g c) -> p g c", c=D_INNER)[:, :nq, :],
                        in1=yu.rearrange("p (g c) -> p g c", c=D_INNER)[:, :nq, :],
                        op=ALU.mult)
                    ve.tensor_reduce(out=ss_st[:, q0:q0 + nq], in_=sq[:, :nq, :],
                                     axis=mybir.AxisListType.X, op=ALU.add)
                # r = (ss/64 + eps)^-1/2 = exp(-0.5*ln(ss/64 + eps))
                nc.scalar.activation(r_st[:, m0:m0 + nm], ss_st[:, m0:m0 + nm],
                                     AF.Ln, bias=eps_s.ap(), scale=1.0 / 64)
                nc.scalar.activation(r_st[:, m0:m0 + nm], r_st[:, m0:m0 + nm],
                                     AF.Exp, scale=-0.5)
                # MM2 + softmax per GO-group
                for h0 in (range(m0, m0 + nm, GO) if debug_stop not in ("b1", "b1x") else []):
                    nh = min(GO, NB - h0)
                    assert nh % 2 == 0
                    GOH = GO // 2
                    for par in range(2):       # 0: even blocks, 1: odd
                        nhp = nh // 2
                        o2 = o2p.tile([128, GOH * D_MODEL], F32)
                        for j in range(nhp):
                            b = h0 + 2 * j + par
                            ytt = yt_tiles[(b - m0) // 2]
                            lhs = ytt[par * 64:par * 64 + 64, :]
                            rhs_w = woutt_s[par * 64:par * 64 + 64, :]
                            nc.tensor.matmul(
                                o2[:, j * D_MODEL:(j + 1) * D_MODEL],
                                lhs, rhs_w)
                        blk_sel = slice(h0 + par, h0 + nh, 2)
                        on = onp.tile([128, GOH, D_MODEL], F32)
                        ve.tensor_tensor(
                            out=on[:, :nhp, :],
                            in0=o2.rearrange("p (g c) -> p g c",
                                             c=D_MODEL)[:, :nhp, :],
                            in1=bcast(r_st[:, blk_sel], D_MODEL),
                            op=ALU.mult)
                        e_t = ep.tile([128, GOH, D_MODEL], F32)
                        nc.scalar.activation(e_t[:, :nhp, :], on[:, :nhp, :],
                                             AF.Exp)
                        se = sep.tile([128, GOH], F32)
                        ve.tensor_reduce(out=se[:, :nhp], in_=e_t[:, :nhp, :],
                                         axis=mybir.AxisListType.X, op=ALU.add)
                        rec = sep.tile([128, GOH], F32)
                        ve.reciprocal(rec[:, :nhp], se[:, :nhp])
                        os_t = osp.tile([128, GOH, D_MODEL], F32)
                        ve.tensor_tensor(out=os_t[:, :nhp, :],
                                         in0=e_t[:, :nhp, :],
                                         in1=bcast(rec[:, :nhp], D_MODEL),
                                         op=ALU.mult)
                        nc.sync.dma_start(
                            out_d.rearrange("(nb p) c -> p nb c", p=128)
                                 [:, blk_sel, :],
                            os_t[:, :nhp, :])
    if debug_stop in ("b1", "b1x"):
        with tile.TileContext(nc) as tc2:
            with tc2.tile_pool(name="zp2", bufs=1) as zp:
                z = zp.tile([128, NB, D_MODEL], F32)
                nc.vector.memset(z, 0.0)
                nc.sync.dma_start(
                    out_d.rearrange("(nb p) c -> p nb c", p=128), z)
    return nc


CTRL_OPS = ("Drain", "NoOp", "Nop", "EventSemaphoreOp", "SemaphoreOp")


def split_overloaded_waits(nc, cap=1, ctrl_only=False):
    n_fixed = 0
    for f in nc.m.functions:
        for bb in f.blocks:
            insts = bb.instructions
            i = 0
            while i < len(insts):
                ins = insts[i]
                si = ins.sync_info
                eff_cap = cap
                if ctrl_only and str(ins.opcode) not in CTRL_OPS:
                    eff_cap = 255
                if si is not None and si.on_wait and len(si.on_wait) > eff_cap:
                    waits = list(si.on_wait)
                    extra, keep = waits[:-cap], waits[-cap:]
                    pos = i
                    for j in range(0, len(extra), cap):
                        chunk = extra[j:j + cap]  # noqa
                        nop = mybir.InstNoOp(
                            name=nc.get_next_instruction_name(), ins=[], outs=[])
                        nop.engine = ins.engine
                        nop.sync_info = mybir.SyncInfo(on_wait=chunk,
                                                       on_update=[])
                        nc.register_instruction(nop)
                        insts.insert(pos, nop)
                        pos += 1
                        i += 1
                    si.on_wait = keep
                    ins.sync_info = si
                    n_fixed += 1
                i += 1
    return n_fixed


def run(x, f_out_w, f_out_b, in_proj_w, conv_w, conv_b, dt_bias, A_log,
        D_skip, norm_w, out_proj_w, num_cores=8, y_engine="vector",
        trace=False, sim_safe=False):
    from concourse.bass_utils import run_bass_kernel_spmd
    N = x.shape[0]
    assert N % (num_cores * 128) == 0
    npc = N // num_cores
    w1t, woutt, db = fold_weights(f_out_w, f_out_b, in_proj_w, conv_w,
                                  conv_b, dt_bias, A_log, D_skip, norm_w,
                                  out_proj_w)
    xt = prep_xt(x)
    nc = build_kernel(npc, num_cores=num_cores, y_engine=y_engine,
                      sim_safe=sim_safe)
    split_overloaded_waits(nc)
    in_maps = []
    for c in range(num_cores):
        in_maps.append({
            "xt": np.ascontiguousarray(xt[:, c * npc:(c + 1) * npc]),
            "w1t": w1t, "woutt": woutt, "db": db,
            "eps": np.full((128, 1), NORM_EPS, np.float32),
            "ident": np.eye(128, dtype=ml_dtypes.bfloat16),
        })
    res = run_bass_kernel_spmd(nc, in_maps, list(range(num_cores)),
                               trace=trace)
    out = np.concatenate([res.results[c]["out"] for c in range(num_cores)],
                         axis=0)
    return out, res


_CACHED = {}


def kernel(x, f_out_w, f_out_b, in_proj_w, conv_w, conv_b, dt_bias, A_log,
           D_skip, norm_w, out_proj_w):
    out, _ = run(x, f_out_w, f_out_b, in_proj_w, conv_w, conv_b, dt_bias,
                 A_log, D_skip, norm_w, out_proj_w, num_cores=8,
                 y_engine="gpsimd" if _CACHED.get("ye") != "vector"
                 else "vector")
    return out.astype(np.float32)



# revision 2
# speedup vs baseline: 1.4825x; 1.4825x over previous
"""AudioMamba2 fused TRN2 kernel v2: 8-core data-parallel Bass/Tile.

Row-major pipeline, bf16 PSUM matmul outputs, big-batch activations,
d-major channel permutation (DVE 2x on the y broadcast-multiply),
PE-based sum-of-squares, DMA-based PSUM->SBUF transpose copies, and
GPSIMD (Pool) offload for bc / f / softmax-reduce / normalize.
"""
import numpy as np
import ml_dtypes

import concourse.bass as bass
import concourse.mybir as mybir
import concourse.tile as tile
from concourse.bass_types import AP

F32 = mybir.dt.float32
BF16 = mybir.dt.bfloat16
AF = mybir.ActivationFunctionType
ALU = mybir.AluOpType

IN_DIM = 36
D_MODEL = 32
D_INNER = 64
NHEADS = 8
D_IN_PROJ = 152
NORM_EPS = 1e-5
K1 = 37          # 36 features + ones row
NSIL = 144       # z|xh|B|C channels (silu'd)

MACRO = 64       # blocks per macro-iteration
GROUPS = (7, 7, 7, 7, 7, 7, 7, 7, 4, 4)   # MM1/silu groups inside a macro


def fold_weights(f_out_w, f_out_b, in_proj_w, conv_w, conv_b, dt_bias,
                 A_log, D_skip, norm_w, out_proj_w):
    f64 = np.float64
    W12 = in_proj_w.astype(f64) @ f_out_w.astype(f64)          # [152, 36]
    b12 = in_proj_w.astype(f64) @ f_out_b.astype(f64)          # [152]
    s80 = conv_w[:, -1].astype(f64)
    W12[64:144] *= s80[:, None]
    b12[64:144] = b12[64:144] * s80 + conv_b.astype(f64)
    b12[144:152] += dt_bias.astype(f64)
    W1 = np.concatenate([W12, b12[:, None]], axis=1)           # [152, 37]
    Wout = out_proj_w.astype(f64) * norm_w.astype(f64)[None, :]  # [32, 64]
    WoutT = np.ascontiguousarray(Wout.T)                       # [64, 32]
    # d-major permutation of the 64 inner channels: new j=(d*8+h) <- old h*8+d
    perm = np.array([(j % 8) * 8 + j // 8 for j in range(64)])
    W1p = W1.copy()
    W1p[0:64] = W1[0:64][perm]          # z block
    W1p[64:128] = W1[64:128][perm]      # xh block
    WoutTp = WoutT[perm]                # rows follow the y channel order
    W1T = np.ascontiguousarray(W1p.T)                          # [37, 152]
    WoutT4 = np.zeros((128, 64), np.float64)   # [[W', 0], [0, W']]
    WoutT4[0:64, 0:32] = WoutTp
    WoutT4[64:128, 32:64] = WoutTp
    return (W1T.astype(ml_dtypes.bfloat16),
            WoutT4.astype(ml_dtypes.bfloat16),
            np.ascontiguousarray(
                np.broadcast_to(D_skip.astype(ml_dtypes.bfloat16),
                                (128, 8))))


def prep_xt(x):
    """x [N, 36] f32 -> xT [37, N] bf16 with ones row."""
    N = x.shape[0]
    xt = np.empty((K1, N), dtype=ml_dtypes.bfloat16)
    xt[:IN_DIM] = x.T.astype(ml_dtypes.bfloat16)
    xt[IN_DIM] = np.float32(1.0)
    return xt


def make_consts():
    essq = np.zeros((128, 8, 16), dtype=ml_dtypes.bfloat16)
    for q in range(8):
        essq[0:64, q, q] = 1.0          # parity 0 rows -> col q
        essq[64:128, q, 8 + q] = 1.0    # parity 1 rows -> col 8+q
    essq = np.ascontiguousarray(essq.reshape(128, 128))
    ident = np.eye(128, dtype=ml_dtypes.bfloat16)
    return essq, ident


def build_kernel(npc, num_cores=8, sim_safe=False):
    assert npc % (MACRO * 128) == 0
    NB = npc // 128
    NM = NB // MACRO
    nc = bass.Bass("TRN2", target_bir_lowering=False, num_devices=num_cores)

    xt_d = nc.dram_tensor("xt", [K1, npc], BF16, kind="ExternalInput")
    w1t_d = nc.dram_tensor("w1t", [K1, D_IN_PROJ], BF16, kind="ExternalInput")
    woutt_d = nc.dram_tensor("woutt", [128, 2 * D_MODEL], BF16,
                             kind="ExternalInput")
    d_d = nc.dram_tensor("dskip", [128, NHEADS], BF16, kind="ExternalInput")
    essq_d = nc.dram_tensor("essq", [128, 128], BF16, kind="ExternalInput")
    id_d = nc.dram_tensor("ident", [128, 128], BF16, kind="ExternalInput")
    idf_d = nc.dram_tensor("identf", [16, 16], F32, kind="ExternalInput")
    eps_d = nc.dram_tensor("eps", [128, 1], F32, kind="ExternalInput")
    out_d = nc.dram_tensor("out", [128, NB * D_MODEL], BF16,
                           kind="ExternalOutput")

    # persistent SBUF constants
    w1t_s = nc.alloc_sbuf_tensor("w1t_s", [K1, D_IN_PROJ], BF16)
    woutt_s = nc.alloc_sbuf_tensor("woutt_s", [128, 2 * D_MODEL], BF16)
    d_s = nc.alloc_sbuf_tensor("d_s", [128, NHEADS], BF16)
    essq_s = nc.alloc_sbuf_tensor("essq_s", [128, 128], BF16)
    id_s = nc.alloc_sbuf_tensor("id_s", [128, 128], BF16)
    idf_s = nc.alloc_sbuf_tensor("idf_s", [16, 16], F32)
    eps_s = nc.alloc_sbuf_tensor("eps_s", [128, 1], F32)

    # manual PSUM map (8 banks exactly)
    pa = [nc.alloc_psum_tensor("pa0", [128, 1024], F32),
          nc.alloc_psum_tensor("pa1", [128, 1024], F32)]    # 2 banks each
    pb = nc.alloc_psum_tensor("pb", [128, 512], F32)        # dt preacts
    ssb = nc.alloc_psum_tensor("ssb", [128, 512], F32)      # ss area + sstr
    ytp = nc.alloc_psum_tensor("ytp", [128, 1024], BF16)    # 2 half-buffers
    o2 = nc.alloc_psum_tensor("o2", [128, 512], F32)        # 2 halves of 8 blk

    ve = nc.vector
    ge = nc.gpsimd

    def silu(out_ap, in_ap):
        if sim_safe:
            nc.scalar.activation(out_ap, in_ap, AF.Sigmoid)
            ve.tensor_tensor(out=out_ap, in0=out_ap, in1=in_ap, op=ALU.mult)
        else:
            nc.scalar.activation(out_ap, in_ap, AF.Silu)

    def sub(ap, off, dims):
        """AP at free-element offset `off` with explicit free dims."""
        return AP(ap.tensor, ap.offset + off, [list(ap.ap[0])] + dims)

    with tile.TileContext(nc) as tc:
        nc.sync.dma_start(w1t_s.ap(), w1t_d.ap())
        nc.sync.dma_start(woutt_s.ap(), woutt_d.ap())
        nc.sync.dma_start(d_s.ap(), d_d.ap())
        nc.sync.dma_start(essq_s.ap(), essq_d.ap())
        nc.sync.dma_start(id_s.ap(), id_d.ap())
        nc.sync.dma_start(idf_s.ap(), idf_d.ap())
        nc.sync.dma_start(eps_s.ap(), eps_d.ap())

        with (
            tc.tile_pool(name="xtp", bufs=3) as xtp,
            tc.tile_pool(name="sp", bufs=4) as sp,
            tc.tile_pool(name="t1p", bufs=2) as t1p,
            tc.tile_pool(name="bcpp", bufs=4) as bcpp,
            tc.tile_pool(name="bcsp", bufs=3) as bcsp,
            tc.tile_pool(name="dtp", bufs=3) as dtp,
            tc.tile_pool(name="fp", bufs=3) as fp,
            tc.tile_pool(name="yup", bufs=8) as yup,
            tc.tile_pool(name="ytsp", bufs=3) as ytsp,
            tc.tile_pool(name="sqp", bufs=4) as sqp,
            tc.tile_pool(name="ssbp", bufs=3) as ssbp,
            tc.tile_pool(name="rp", bufs=3) as rp,
            tc.tile_pool(name="onp", bufs=3) as onp,
            tc.tile_pool(name="ep", bufs=3) as ep,
            tc.tile_pool(name="sep", bufs=4) as sep,
            tc.tile_pool(name="osp", bufs=2) as osp,
        ):
            prev = None     # state of macro m-1, C phase still pending

            def emit_mm2_group(st, hh):
                """MM2s for 8 blocks + the r-scale multiply (Pool)."""
                oh = (hh % 2) * 256
                yts_p, r_p, on_p = st["yts"], st["r"], st["on"]
                for pair in range(4):
                    lhs = yts_p[:, hh * 512 + pair * 128:
                                hh * 512 + (pair + 1) * 128]
                    nc.tensor.matmul(
                        o2[:, oh + pair * 64:oh + (pair + 1) * 64],
                        lhs, woutt_s.ap())
                # o2 cols (pair, par, ch); r column u = 16c + 8par + q
                ge.tensor_tensor(
                    out=sub(on_p, hh * 256, [[32, 8], [1, 32]]),
                    in0=sub(o2.ap(), oh, [[64, 4], [32, 2], [1, 32]]),
                    in1=sub(r_p, hh, [[16, 4], [8, 2], [0, 32]]),
                    op=ALU.mult)

            def emit_c_exp(st):
                """exp + softmax sum for macro st."""
                e_t = ep.tile([128, MACRO * D_MODEL], F32)
                nc.scalar.activation(e_t, st["on"], AF.Exp)
                se_t = sep.tile([128, MACRO], F32)
                ve.tensor_reduce(
                    out=se_t,
                    in_=sub(e_t, 0, [[32, MACRO], [1, 32]]),
                    axis=mybir.AxisListType.X, op=ALU.add)
                st["e"], st["se"] = e_t, se_t

            def emit_c_norm(st):
                """normalize + output DMA for macro st (off critical path)."""
                rec_t = sep.tile([128, MACRO], F32)
                ve.reciprocal(rec_t, st["se"])
                os_t = osp.tile([128, MACRO * D_MODEL], BF16)
                ge.tensor_tensor(
                    out=os_t, in0=st["e"],
                    in1=sub(rec_t, 0, [[1, MACRO], [0, D_MODEL]]),
                    op=ALU.mult)
                nc.sync.dma_start(
                    out_d[:, st["mb0"] * D_MODEL:
                          (st["mb0"] + MACRO) * D_MODEL],
                    os_t)

            for m in range(NM + 1):
                if m < NM:
                    mb0 = m * MACRO
                    xt_t = xtp.tile([K1, MACRO * 128], BF16)
                    nc.sync.dma_start(xt_t,
                                      xt_d[:, mb0 * 128:(mb0 + MACRO) * 128])
                    t1_t = t1p.tile([128, MACRO * D_INNER], BF16)
                    bc_t = bcsp.tile([128, MACRO], F32)

                    # ---- A(m) with C(m-1) MM2 groups interleaved ----
                    g0 = 0
                    for gi, G in enumerate(GROUPS):
                        pa_t = pa[gi % 2]
                        for j in range(G):
                            b = g0 + j
                            xt_sl = xt_t[:, b * 128:(b + 1) * 128]
                            off = j * NSIL
                            # split MM1 outputs crossing the f32 PSUM bank
                            # boundary at element 512
                            if off < 512 < off + NSIL:
                                cut = 512 - off
                                nc.tensor.matmul(pa_t[:, off:512],
                                                 xt_sl, w1t_s[:, 0:cut])
                                nc.tensor.matmul(pa_t[:, 512:off + NSIL],
                                                 xt_sl, w1t_s[:, cut:NSIL])
                            else:
                                nc.tensor.matmul(pa_t[:, off:off + NSIL],
                                                 xt_sl, w1t_s[:, 0:NSIL])
                            nc.tensor.matmul(pb[:, b * 8:b * 8 + 8],
                                             xt_sl, w1t_s[:, NSIL:D_IN_PROJ])
                        if prev is not None and gi < 8:
                            emit_mm2_group(prev, gi)
                        s_t = sp.tile([128, 1008], BF16)
                        silu(sub(s_t, 0, [[NSIL, G], [1, NSIL]]),
                             sub(pa_t.ap(), 0, [[NSIL, G], [1, NSIL]]))
                        # t1 = S_z * S_xh  (DVE, bf16 2x)
                        ve.tensor_tensor(
                            out=sub(t1_t, g0 * D_INNER,
                                    [[D_INNER, G], [1, D_INNER]]),
                            in0=sub(s_t, 0, [[NSIL, G], [1, 64]]),
                            in1=sub(s_t, 64, [[NSIL, G], [1, 64]]),
                            op=ALU.mult)
                        # bc = sum(S_B * S_C)
                        bcp_t = bcpp.tile([128, 14 * 8], BF16)
                        ve.tensor_tensor(
                            out=sub(bcp_t, 0, [[8, G], [1, 8]]),
                            in0=sub(s_t, 128, [[NSIL, G], [1, 8]]),
                            in1=sub(s_t, 136, [[NSIL, G], [1, 8]]),
                            op=ALU.mult)
                        ve.tensor_reduce(
                            out=sub(bc_t, g0, [[1, G]]),
                            in_=sub(bcp_t, 0, [[8, G], [1, 8]]),
                            axis=mybir.AxisListType.X, op=ALU.add)
                        g0 += G

                    # ---- C(m-1): exp + softmax sum ----
                    if prev is not None:
                        emit_c_exp(prev)

                    # ---- dt path: softplus + f = dt*bc + D ----
                    dt_t = dtp.tile([128, MACRO * NHEADS], BF16)
                    nc.scalar.activation(dt_t, pb.ap(), AF.Exp)
                    nc.scalar.activation(dt_t, dt_t, AF.Ln, bias=1.0)
                    f_t = fp.tile([128, MACRO * NHEADS], BF16)
                    ve.tensor_tensor(
                        out=f_t, in0=dt_t,
                        in1=sub(bc_t, 0, [[1, MACRO], [0, NHEADS]]),
                        op=ALU.mult)
                    ve.tensor_tensor(
                        out=f_t, in0=f_t,
                        in1=sub(d_s.ap(), 0, [[0, MACRO], [1, NHEADS]]),
                        op=ALU.add)

                    # ---- B(m): y, transpose, copy-out, ss ----
                    yts_t = ytsp.tile([128, MACRO * D_INNER], BF16)
                    yu_ts = []
                    for q in range(8):
                        yu_t = yup.tile([128, 512], BF16)
                        ve.tensor_tensor(
                            out=yu_t,
                            in0=t1_t[:, q * 512:(q + 1) * 512],
                            in1=sub(f_t, q * 64,
                                    [[NHEADS, 8], [0, 8], [1, NHEADS]]),
                            op=ALU.mult)
                        yu_ts.append(yu_t)
                    sq_ts = {}

                    def emit_bq(q):
                        yh = (q % 2) * 512
                        sq_t = sqp.tile([128, 512], BF16)
                        ve.tensor_tensor(out=sq_t,
                                         in0=ytp[:, yh:yh + 512],
                                         in1=ytp[:, yh:yh + 512],
                                         op=ALU.mult)
                        cp = ve if q < 2 else ge
                        cp.tensor_copy(yts_t[:, q * 512:(q + 1) * 512],
                                       ytp[:, yh:yh + 512])
                        sq_ts[q] = sq_t

                    def emit_ssred(q):
                        nc.tensor.matmul(ssb[0:16, 0:512],
                                         essq_s[:, q * 16:(q + 1) * 16],
                                         sq_ts.pop(q),
                                         start=(q == 0), stop=(q == 7))

                    for q in range(8):
                        yh = (q % 2) * 512
                        for p in range(4):
                            nc.tensor.transpose(
                                ytp[:, yh + p * 128:yh + (p + 1) * 128],
                                yu_ts[q][:, p * 128:(p + 1) * 128],
                                id_s.ap())
                        if q > 0:
                            emit_ssred(q - 1)
                        emit_bq(q)
                    emit_ssred(7)
                    # ss -> SBUF -> row-major -> r
                    ssb_t = ssbp.tile([16, 512], F32)
                    ve.tensor_copy(ssb_t, ssb[0:16, 0:512])
                    for c in range(4):
                        nc.tensor.transpose(
                            ssb[:, 16 * c:16 * (c + 1)],
                            ssb_t[0:16, 128 * c:128 * (c + 1)],
                            idf_s.ap())
                    r_t = rp.tile([128, MACRO], F32)
                    nc.scalar.activation(r_t, ssb[:, 0:64], AF.Ln,
                                         scale=1.0 / 64, bias=eps_s.ap())
                    nc.scalar.activation(r_t, r_t, AF.Exp, scale=-0.5)
                    on_t = onp.tile([128, MACRO * D_MODEL], F32)
                    if prev is not None:
                        emit_c_norm(prev)
                    prev = {"mb0": mb0, "yts": yts_t, "r": r_t, "on": on_t}
                else:
                    # trailing C for the final macro
                    for hh in range(8):
                        emit_mm2_group(prev, hh)
                    emit_c_exp(prev)
                    emit_c_norm(prev)
    return nc


def split_overloaded_waits(nc, cap=1):
    n_fixed = 0
    for f in nc.m.functions:
        for bb in f.blocks:
            insts = bb.instructions
            i = 0
            while i < len(insts):
                ins = insts[i]
                si = ins.sync_info
                if si is not None and si.on_wait and len(si.on_wait) > cap:
                    waits = list(si.on_wait)
                    extra, keep = waits[:-cap], waits[-cap:]
                    pos = i
                    for j in range(0, len(extra), cap):
                        chunk = extra[j:j + cap]
                        nop = mybir.InstNoOp(
                            name=nc.get_next_instruction_name(), ins=[],
                            outs=[])
                        nop.engine = ins.engine
                        nop.sync_info = mybir.SyncInfo(on_wait=chunk,
                                                       on_update=[])
                        nc.register_instruction(nop)
                        insts.insert(pos, nop)
                        pos += 1
                        i += 1
                    si.on_wait = keep
                    ins.sync_info = si
                    n_fixed += 1
                i += 1
    return n_fixed


def run(x, f_out_w, f_out_b, in_proj_w, conv_w, conv_b, dt_bias, A_log,
        D_skip, norm_w, out_proj_w, num_cores=8, trace=False, sim_safe=False):
    from concourse.bass_utils import run_bass_kernel_spmd
    N = x.shape[0]
    assert N % (num_cores * MACRO * 128) == 0
    npc = N // num_cores
    NB = npc // 128
    w1t, woutt2, dsk = fold_weights(f_out_w, f_out_b, in_proj_w, conv_w,
                                    conv_b, dt_bias, A_log, D_skip, norm_w,
                                    out_proj_w)
    xt = prep_xt(x)
    essq, ident = make_consts()
    identf = np.eye(16, dtype=np.float32)
    nc = build_kernel(npc, num_cores=num_cores, sim_safe=sim_safe)
    split_overloaded_waits(nc)
    in_maps = []
    for c in range(num_cores):
        in_maps.append({
            "xt": np.ascontiguousarray(xt[:, c * npc:(c + 1) * npc]),
            "w1t": w1t, "woutt": woutt2, "dskip": dsk,
            "essq": essq, "ident": ident, "identf": identf,
            "eps": np.full((128, 1), NORM_EPS, np.float32),
        })
    res = run_bass_kernel_spmd(nc, in_maps, list(range(num_cores)),
                               trace=trace)
    outs = []
    for c in range(num_cores):
        o = np.asarray(res.results[c]["out"]).reshape(128, NB, D_MODEL)
        outs.append(np.ascontiguousarray(o.transpose(1, 0, 2))
                    .reshape(npc, D_MODEL))
    return np.concatenate(outs, axis=0).astype(np.float32), res


def kernel(x, f_out_w, f_out_b, in_proj_w, conv_w, conv_b, dt_bias, A_log,
           D_skip, norm_w, out_proj_w):
    out, _ = run(x, f_out_w, f_out_b, in_proj_w, conv_w, conv_b, dt_bias,
                 A_log, D_skip, norm_w, out_proj_w, num_cores=8)
    return out
